# revision 24
# baseline (speedup 1.0000x reference)
"""GATv2 layer kernel for Trainium2 (Bass/Tile), 8-core SPMD.

Problem (hardcoded): B=4, N=512, D=128, H=8 heads, F=16 hidden, is_concat.
  g_l = h @ W_l.T ; g_r = h @ W_r.T               [B,N,H,F]
  e[b,i,j,h] = sum_f a_w[f]*lrelu(g_l[b,j,h,f] + g_r[b,i,h,f], 0.2)
  a = softmax_j(e masked by adj)                  [B,i,j,H]
  out[b,i,h,f] = sum_j a[b,i,j,h]*g_r[b,j,h,f]   -> [B,N,H*F]

Sharding: 8 cores = (batch b in 0..4) x (i-half in 0..2). Each core handles
256 target nodes i of one batch with fully-replicated g_l/g_r.

Math used on device (per core, b fixed):
  lrelu(x) = 0.8*relu(x) + 0.2*x, so
  e[i,j,h] = 0.8*sum_hf A[hf,h]*relu(g_lT[hf,j] + g_rT[hf,i]) + 0.2*alT[h,j]
             + 0.2*ar[i,h]
  The per-(i,h) additive term 0.2*ar cancels in softmax (shift invariance),
  so it is dropped. The adjacency mask is folded into the scores as an
  additive +30*mask accumulated by the same PSUM matmul that adds the
  0.2*alT linear term (K=24 stationary selector); exp then runs with bias
  -30: unmasked j recover exp(e) exactly, masked j give exp(e-30) which
  underflows fp16 to exact 0. Softmax normalization is applied after
  aggregation (linearity), with the denominator produced for free by an
  all-ones column appended to g_r.

Per group of 16 i's (PSUM partitions = (i_local*8 + h)):
  M[24,512]     = [0.2*alT (8 rows); 30*mask rows (16)]   DMA-filled
  psum          = Wlm.T @ M  (start)                      PE  (K=24)
  t[hf, j]      = relu(g_lT + g_rT[:, i] bias)            DVE/ACT/GPSIMD
  psum += A_aw.T @ t  (M=64 strips, alternating halves)   PE
  u, den        = exp(psum - 30), rowsum via accum_out    ACT (reads PSUM)
  uT            = dma transpose (16x128 XBAR tiles)       DMA engines
  agg_ps        = sum_j uT.T @ g_r_nat (4 K-chunks)       PE
  rd            = 1/den                                   DVE
  agg_sb        = agg_ps * rd * headmask                  DVE (fused)
  out[i,hf]     = R.T @ agg_sb  (sum over h)              PE
The exp/transpose of group g and the aggregation of group g-1 are emitted
one/two iterations late (software pipelining): each engine's queue is
in-order, so this keeps PE streaming scores and ACT streaming relus with
no head-of-line blocking on cross-engine dependencies.
"""

import ml_dtypes
import numpy as np
from contextlib import ExitStack

import concourse.bass as bass
import concourse.bacc as bacc
import concourse.tile as tile
import concourse.mybir as mybir
from concourse.bass_utils import run_bass_kernel_spmd

B, N, D = 4, 512, 128
H, F = 8, 16
NEG_SLOPE = 0.2
NCORES = 8
IHALF = N // 2          # 256 target nodes per core
GSIZE = 16              # i's per group
NGROUPS = IHALF // GSIZE  # 16
MBIG = 30.0             # mask boost; exp(e-30) underflows fp16 to exact 0
f32 = mybir.dt.float32
f16 = mybir.dt.float16

# The score path (relu'd pairwise features t and the per-node score matmuls)
# runs in fp16: full 1-cycle/row PE streaming and DVE packed 16-bit modes --
# with a 10-bit mantissa and ample range for these tiny values. The noise
# lands only in pre-softmax scores; softmax normalization and the
# aggregation path stay exact fp32.

# Which of the 16 per-group relu ops run on ScalarE (the rest on VectorE:
# 347ns vs 710ns per op measured). ScalarE gets the LAST-consumed score
# positions so PE tracks VectorE's just-in-time stream first while ScalarE
# works ahead. GpSimd is useless for bulk elementwise work (software
# TensorScalar measures ~7.5us per [128,512] op) -- it only dispatches DMAs.
ACT_RELU_KS = (13, 6, 14, 7, 15)


def build_program():
    nc = bacc.Bacc(
        "TRN2", target_bir_lowering=False, debug=False, num_devices=NCORES
    )

    d_hT = nc.dram_tensor("hT", [D, N], f16, kind="ExternalInput").ap()
    d_WlT = nc.dram_tensor("WlT", [D, H * F], f16, kind="ExternalInput").ap()
    d_WrT = nc.dram_tensor("WrT", [D, H * F], f16, kind="ExternalInput").ap()
    # Amask[:, 56:64] is 0.8*A_aw (zeros elsewhere): 16 accumulating M=64
    # matmuls (8 per 64-row half, alternating PE column strips so LDWEIGHTS
    # overlaps the other half's in-flight matmul) compose 16 target nodes
    # into one full-height PSUM tile. lrelu = 0.8*relu + 0.2*identity; the
    # 0.8 is folded into these weights.
    d_Aaw = nc.dram_tensor("Amask", [H * F, 120], f16, kind="ExternalInput").ap()
    d_Aaw02 = nc.dram_tensor("Aaw02", [H * F, H], f16, kind="ExternalInput").ap()
    # Wal [8,128] replicates the 0.2*alT linear term over i_local (select
    # h = p%8); Wm [16,128] replicates 30*mask rows over h (select
    # il = p//8). Two K<=16 matmuls seed each group's score PSUM with
    # lin-term + mask boost.
    d_Wal = nc.dram_tensor("Wal", [H, 128], f16, kind="ExternalInput").ap()
    d_Wm = nc.dram_tensor("Wm", [GSIZE, 128], f16, kind="ExternalInput").ap()
    # Rmask[:, 64q:64q+64] holds the head-sum reduction matrix placed in
    # columns [16q:16q+16] (4 accumulating matmuls -> one 64-row PSUM stripe).
    d_R = nc.dram_tensor("Rmask", [128, 4 * 64], f16, kind="ExternalInput").ap()
    d_hm = nc.dram_tensor("headmask", [128, H * F], f32, kind="ExternalInput").ap()
    # mask30[il, 512g + j] = 30*adj row for target node 16g+il: all 16
    # groups' mask blocks live in one [16, 16*512] tile, loaded ONCE at
    # setup (per-group DMAs would couple the transpose chain to the slow
    # SWDGE queue via framework DMA-ordering fences).
    d_mask = nc.dram_tensor("mask30", [GSIZE, NGROUPS * N], f16,
                            kind="ExternalInput").ap()
    d_out = nc.dram_tensor("out", [IHALF, D], f32, kind="ExternalOutput").ap()

    with tile.TileContext(nc) as tc:
        with ExitStack() as ctx:
            _gat_body(ctx, tc, d_out, d_hT, d_WlT, d_WrT, d_Aaw, d_Aaw02,
                      d_Wal, d_Wm, d_R, d_hm, d_mask)
    nc.compile()
    return nc


def _gat_body(ctx, tc, d_out, d_hT, d_WlT, d_WrT, d_Aaw, d_Aaw02, d_Wal,
              d_Wm, d_R, d_hm, d_mask):
    nc = tc.nc
    add = mybir.AluOpType.add
    mult = mybir.AluOpType.mult
    amax = mybir.AluOpType.max
    Relu = mybir.ActivationFunctionType.Relu
    Exp = mybir.ActivationFunctionType.Exp

    consts = ctx.enter_context(tc.tile_pool(name="consts", bufs=1))
    tpool = ctx.enter_context(tc.tile_pool(name="tpool", bufs=24))
    upool = ctx.enter_context(tc.tile_pool(name="upool", bufs=5))
    utpool = ctx.enter_context(tc.tile_pool(name="utpool", bufs=7))
    aggp = ctx.enter_context(tc.tile_pool(name="aggp", bufs=5))
    denp = ctx.enter_context(tc.tile_pool(name="denp", bufs=6))
    outp = ctx.enter_context(tc.tile_pool(name="outp", bufs=2))

    ppe = ctx.enter_context(tc.tile_pool(name="ppe", bufs=3, space="PSUM"))
    pagg = ctx.enter_context(tc.tile_pool(name="pagg", bufs=4, space="PSUM"))
    pout = ctx.enter_context(tc.tile_pool(name="pout", bufs=1, space="PSUM"))

    # ---- load constants (hT/W on the critical path first) ----
    s_hT = consts.tile([D, N], f16, tag="ht")
    nc.sync.dma_start(out=s_hT[:], in_=d_hT)
    s_WlT = consts.tile([D, H * F], f16, tag="wlt")
    nc.scalar.dma_start(out=s_WlT[:], in_=d_WlT)
    s_WrT = consts.tile([D, H * F], f16, tag="wrt")
    nc.scalar.dma_start(out=s_WrT[:], in_=d_WrT)
    s_Aaw = consts.tile([H * F, 120], f16, tag="aaw")
    nc.gpsimd.dma_start(out=s_Aaw[:], in_=d_Aaw)
    s_Aaw02 = consts.tile([H * F, H], f16, tag="aaw02")
    nc.gpsimd.dma_start(out=s_Aaw02[:], in_=d_Aaw02)
    s_Wal = consts.tile([H, 128], f16, tag="wal")
    nc.gpsimd.dma_start(out=s_Wal[:], in_=d_Wal)
    s_Wm = consts.tile([GSIZE, 128], f16, tag="wm")
    nc.gpsimd.dma_start(out=s_Wm[:], in_=d_Wm)
    s_mask = consts.tile([GSIZE, NGROUPS * N], f16, tag="mask")
    nc.gpsimd.dma_start(out=s_mask[:], in_=d_mask)
    s_R = consts.tile([128, 4 * 64], f16, tag="rmat")
    nc.gpsimd.dma_start(out=s_R[:], in_=d_R)
    s_hm = consts.tile([128, H * F], f32, tag="hm")
    nc.gpsimd.dma_start(out=s_hm[:], in_=d_hm)
    neg30 = consts.tile([128, 1], f32, tag="neg30")
    nc.vector.memset(neg30[:], -MBIG)

    # ---- setup: projections ----
    # g_lT[hf, j] = sum_d WlT[d, hf] * hT[d, j]  (fp16: feeds the fp16
    # score path only)
    g_lT = consts.tile([H * F, N], f16, tag="glt")
    ps = ppe.tile([128, N], f32, tag="pe")
    nc.tensor.matmul(ps[:], s_WlT[:], s_hT[:], start=True, stop=True)
    nc.scalar.copy(g_lT[:], ps[:])

    g_rT = consts.tile([H * F, N], f32, tag="grt")
    ps = ppe.tile([128, N], f32, tag="pe")
    nc.tensor.matmul(ps[:], s_WrT[:], s_hT[:], start=True, stop=True)
    nc.vector.tensor_copy(g_rT[:], ps[:])

    # 0.2*alT[h, j] = sum_hf 0.2*Aaw[hf, h] * g_lT[hf, j]  -> fp16 [8, 512]
    alsb = consts.tile([H, N], f16, tag="alsb")
    psl = pagg.tile([H, N], f32, tag="agg")
    nc.tensor.matmul(psl[:], s_Aaw02[:], g_lT[:], start=True, stop=True)
    nc.vector.tensor_copy(alsb[:], psl[:])

    # g_r natural layout with an appended ones column per 128-row chunk:
    # chunk c occupies cols [129c, 129c+129); col 129c+128 is all-ones so the
    # aggregation matmul also produces the softmax denominator for free.
    g_r_nat = consts.tile([128, 4 * 129], f16, tag="grnat")
    for c in range(4):
        cs = slice(128 * c, 128 * (c + 1))
        pq = pagg.tile([128, 129], f32, tag="agg")
        nc.tensor.matmul(pq[:, 0:128], s_hT[:, cs], s_WrT[:],
                         start=True, stop=True)
        nc.vector.tensor_copy(g_r_nat[:, 129 * c:129 * c + 128], pq[:, 0:128])
        nc.vector.memset(g_r_nat[:, 129 * c + 128:129 * c + 129], 1.0)

    # ---- main loop: one group of 16 target nodes per iteration ----
    # Stage S(g): relus + score matmuls for group g (emitted in iteration g)
    # Stage E(g): exp + den + dma-transpose     (emitted in iteration g+1)
    # Stage P(g): agg, 1/den, normalize, output (emitted in iteration g+2)
    # visit halves alternately (0,8,1,9,...) so consecutive matmuls hit
    # different PE column strips and weight loads overlap compute
    order = [x for p_ in zip(range(8), range(8, 16)) for x in p_]
    state = {"out_ps": None}

    def escore(g):
        e_ps = ppe.tile([128, N], f32, tag="pe")
        # seed PSUM with lin term (K=8, static moving) + mask boost (K=16,
        # a column block of the preloaded mask tile)
        nc.tensor.matmul(e_ps[:], s_Wal[:], alsb[:], start=True,
                         stop=False, skip_group_check=True)
        nc.tensor.matmul(e_ps[:], s_Wm[:], s_mask[:, N * g:N * (g + 1)],
                         start=False, stop=False, skip_group_check=True)
        for n_, k in enumerate(order):
            i = GSIZE * g + k
            t_t = tpool.tile([H * F, N], f16, tag="t")
            if k in ACT_RELU_KS:
                nc.scalar.activation(t_t[:], g_lT[:], Relu,
                                     bias=g_rT[:, i:i + 1], scale=1.0)
            else:
                # (g_lT + bias) max 0.0, one DVE pass (packed 16-bit mode)
                nc.vector.tensor_scalar(t_t[:], g_lT[:], g_rT[:, i:i + 1],
                                        0.0, add, amax)
            half = 64 * (k // 8)
            w0 = 56 - 8 * (k % 8)
            nc.tensor.matmul(e_ps[half:half + 64, :],
                             s_Aaw[:, w0:w0 + 64], t_t[:],
                             start=False, stop=(n_ >= len(order) - 2),
                             tile_position=(0, half),
                             skip_group_check=True)
        return e_ps

    def expt(e_ps):
        # u = exp(e - 30): unmasked j give exp(e) exactly; masked j
        # underflow fp16 to 0
        u = upool.tile([128, N], f16, tag="u")
        nc.scalar.activation(u[:], e_ps[:], Exp, bias=neg30[:])
        # transpose u via the DMA XBAR (one 16x128-tiled instruction):
        # uT[p, c, m] = u[m, 128c+p], so uT[:, c, :] is the aggregation
        # stationary for source-node chunk c.
        uT = utpool.tile([128, 4, 128], f16, tag="ut")
        nc.sync.dma_start_transpose(out=uT[:, :, :], in_=u[:])
        return (uT,)

    def post_pe(g, uT):
        # aggT[(il,h), hf] = sum_j u[(il,h), j] * g_r[j, hf]; the ones
        # column of g_r_nat yields den = sum_j u in col 128.
        agg_ps = pagg.tile([128, 129], f32, tag="agg")
        for c in range(4):
            nc.tensor.matmul(agg_ps[:], uT[:, c, :],
                             g_r_nat[:, 129 * c:129 * c + 129],
                             start=(c == 0), stop=(c == 3))
        return agg_ps

    def post_dve(g, agg_ps):
        if g % 8 == 0:
            out_ps_t = pout.tile([128, D], f32, tag="out")
            state["out_ps"] = out_ps_t
        out_ps = state["out_ps"]
        rd = denp.tile([128, 1], f32, tag="rden")
        nc.vector.reciprocal(rd[:], agg_ps[:, 128:129])

        # normalize rows by 1/den, keep only the matching head block
        agg_sb = aggp.tile([128, D], f16, tag="aggsb")
        nc.vector.scalar_tensor_tensor(
            agg_sb[:], agg_ps[:, 0:128], rd[:], s_hm[:], mult, mult)

        # out[16q + il, hf] = sum_h agg_sb[(il,h), hf]; 4 groups
        # accumulate into a 64-row stripe via zero-masked weights.
        q = g % 4
        stripe = 64 * ((g % 8) // 4)
        nc.tensor.matmul(out_ps[stripe:stripe + 64, :],
                         s_R[:, 64 * q:64 * q + 64], agg_sb[:],
                         start=(q == 0), stop=(q == 3))
        if q == 3:
            outb = outp.tile([64, D], f32, tag="outb")
            nc.scalar.copy(outb[:], out_ps[stripe:stripe + 64, :])
            r0 = 64 * (g // 4)
            nc.gpsimd.dma_start(out=d_out[r0:r0 + 64, :], in_=outb[:])

    # S(g) in iteration g, E(g) in g+1, agg(g) in g+4, norm/out(g) in g+5:
    # the exp -> dma-transpose -> agg chain costs ~5us (queue + xbar +
    # ~1.2us DMA semaphore propagation), so agg gets three group-periods
    # of slack, and the reciprocal/normalize always find their agg done
    # (no head-of-line blocking of the next group's relus on VectorE).
    eq = []   # [(g, e_ps)] awaiting E
    tq = []   # [(g, uT)] awaiting agg
    aq = []   # [(g, agg_ps)] awaiting norm/out
    for g in range(NGROUPS):
        e_ps = escore(g)
        if eq:
            ge, pe_ = eq.pop(0)
            tq.append((ge, *expt(pe_)))
        if len(tq) > 3:
            ga, uT = tq.pop(0)
            aq.append((ga, post_pe(ga, uT)))
        if len(aq) > 3:
            post_dve(*aq.pop(0))
        eq.append((g, e_ps))

    ge, pe_ = eq.pop(0)
    tq.append((ge, *expt(pe_)))
    while tq or aq:
        if tq:
            ga, uT = tq.pop(0)
            aq.append((ga, post_pe(ga, uT)))
        if aq:
            post_dve(*aq.pop(0))


def _host_inputs(h, adj, W_l, W_r, a_w):
    """Build the per-core input maps (pure layout/constant prep)."""
    HF = H * F
    Aaw = np.zeros((HF, H), dtype=np.float32)
    for hh in range(H):
        Aaw[hh * F:(hh + 1) * F, hh] = a_w
    Amask = np.zeros((HF, 120), dtype=np.float32)
    Amask[:, 56:64] = (1.0 - NEG_SLOPE) * Aaw
    Amask = Amask.astype(np.float16)
    Aaw02 = (NEG_SLOPE * Aaw).astype(np.float16)
    Wal = np.zeros((H, 128), dtype=np.float16)
    Wm = np.zeros((GSIZE, 128), dtype=np.float16)
    for p in range(128):
        Wal[p % H, p] = 1.0          # lin term: select h
        Wm[p // H, p] = 1.0          # mask boost: select i_local
    Rmask = np.zeros((128, 4 * 64), dtype=np.float16)
    for q in range(4):
        for il in range(GSIZE):
            Rmask[il * H:(il + 1) * H, 64 * q + 16 * q + il] = 1.0
    headmask = np.zeros((128, HF), dtype=np.float32)
    for il in range(GSIZE):
        for hh in range(H):
            headmask[il * H + hh, hh * F:(hh + 1) * F] = 1.0
    WlT = np.ascontiguousarray(W_l.T).astype(np.float16)
    WrT = np.ascontiguousarray(W_r.T).astype(np.float16)

    in_maps = []
    for c in range(NCORES):
        b = c // 2
        i0 = IHALF * (c % 2)
        # Roll the node axis so this core's target nodes sit at positions
        # 0..IHALF-1 (the SPMD program indexes g_rT bias columns by local i).
        # Source-node order is permuted consistently everywhere (softmax and
        # aggregation are permutation-invariant over j).
        in_maps.append({
            "hT": np.ascontiguousarray(np.roll(h[b], -i0, axis=0).T).astype(
                np.float16),
            "WlT": WlT,
            "WrT": WrT,
            "Amask": Amask,
            "Aaw02": Aaw02,
            "Wal": Wal,
            "Wm": Wm,
            "Rmask": Rmask,
            "headmask": headmask,
            "mask30": np.ascontiguousarray(
                (MBIG * np.roll(adj[b, i0:i0 + IHALF, :, 0], -i0, axis=1))
                .reshape(NGROUPS, GSIZE, N).transpose(1, 0, 2)
                .reshape(GSIZE, NGROUPS * N)).astype(np.float16),
        })
    return in_maps


_NC_CACHE = {}
LAST_RESULT = None  # BassKernelResults of the most recent kernel() call


def _get_program():
    if "nc" not in _NC_CACHE:
        _NC_CACHE["nc"] = build_program()
    return _NC_CACHE["nc"]


def kernel(h, adj, W_l, W_r, a_w):
    h = np.asarray(h)
    adj = np.asarray(adj)
    W_l = np.asarray(W_l)
    W_r = np.asarray(W_r)
    a_w = np.asarray(a_w)

    nc = _get_program()
    in_maps = _host_inputs(h, adj, W_l, W_r, a_w)
    res = None
    for attempt in range(3):
        try:
            res = run_bass_kernel_spmd(nc, in_maps, list(range(NCORES)))
            break
        except Exception:
            # the axon-proxied device occasionally reports a transient
            # "unrecoverable" state at process start; it self-heals
            if attempt == 2:
                raise
            import time
            time.sleep(20)
    global LAST_RESULT
    LAST_RESULT = res

    out = np.zeros((B, N, D), dtype=np.float32)
    for c in range(NCORES):
        b = c // 2
        i0 = IHALF * (c % 2)
        out[b, i0:i0 + IHALF, :] = res.results[c]["out"]
    return out


# revision 27
# speedup vs baseline: 1.0960x; 1.0960x over previous
"""GATv2 layer kernel for Trainium2 (Bass/Tile), 8-core SPMD.

Problem (hardcoded): B=4, N=512, D=128, H=8 heads, F=16 hidden, is_concat.
  g_l = h @ W_l.T ; g_r = h @ W_r.T               [B,N,H,F]
  e[b,i,j,h] = sum_f a_w[f]*lrelu(g_l[b,j,h,f] + g_r[b,i,h,f], 0.2)
  a = softmax_j(e masked by adj)                  [B,i,j,H]
  out[b,i,h,f] = sum_j a[b,i,j,h]*g_r[b,j,h,f]   -> [B,N,H*F]

Sharding: 8 cores = (batch b in 0..4) x (i-half in 0..2). Each core handles
256 target nodes i of one batch with fully-replicated g_l/g_r.

Math used on device (per core, b fixed):
  lrelu(x) = 0.8*relu(x) + 0.2*x, so
  e[i,j,h] = 0.8*sum_hf A[hf,h]*relu(g_lT[hf,j] + g_rT[hf,i]) + 0.2*alT[h,j]
             + 0.2*ar[i,h]
  The per-(i,h) additive term 0.2*ar cancels in softmax (shift invariance),
  so it is dropped. The adjacency mask is folded into the scores as an
  additive +30*mask accumulated by the same PSUM matmul that adds the
  0.2*alT linear term (K=24 stationary selector); exp then runs with bias
  -30: unmasked j recover exp(e) exactly, masked j give exp(e-30) which
  underflows fp16 to exact 0. Softmax normalization is applied after
  aggregation (linearity), with the denominator produced for free by an
  all-ones column appended to g_r.

Per group of 16 i's (PSUM partitions = (i_local*8 + h)):
  M[24,512]     = [0.2*alT (8 rows); 30*mask rows (16)]   DMA-filled
  psum          = Wlm.T @ M  (start)                      PE  (K=24)
  t[hf, j]      = relu(g_lT + g_rT[:, i] bias)            DVE/ACT/GPSIMD
  psum += A_aw.T @ t  (M=64 strips, alternating halves)   PE
  u, den        = exp(psum - 30), rowsum via accum_out    ACT (reads PSUM)
  uT            = dma transpose (16x128 XBAR tiles)       DMA engines
  agg_ps        = sum_j uT.T @ g_r_nat (4 K-chunks)       PE
  rd            = 1/den                                   DVE
  agg_sb        = agg_ps * rd * headmask                  DVE (fused)
  out[i,hf]     = R.T @ agg_sb  (sum over h)              PE
The exp/transpose of group g and the aggregation of group g-1 are emitted
one/two iterations late (software pipelining): each engine's queue is
in-order, so this keeps PE streaming scores and ACT streaming relus with
no head-of-line blocking on cross-engine dependencies.
"""

import ml_dtypes
import numpy as np
from contextlib import ExitStack

import concourse.bass as bass
import concourse.bacc as bacc
import concourse.tile as tile
import concourse.mybir as mybir
from concourse.bass_utils import run_bass_kernel_spmd

B, N, D = 4, 512, 128
H, F = 8, 16
NEG_SLOPE = 0.2
NCORES = 8
IHALF = N // 2          # 256 target nodes per core
GSIZE = 16              # i's per group
NGROUPS = IHALF // GSIZE  # 16
MBIG = 30.0             # mask boost; exp(e-30) underflows fp16 to exact 0
f32 = mybir.dt.float32
f16 = mybir.dt.float16

# The score path (relu'd pairwise features t and the per-node score matmuls)
# runs in fp16: full 1-cycle/row PE streaming and DVE packed 16-bit modes --
# with a 10-bit mantissa and ample range for these tiny values. The noise
# lands only in pre-softmax scores; softmax normalization and the
# aggregation path stay exact fp32.

# Which of the 16 per-group relu ops run on ScalarE (the rest on VectorE:
# 347ns vs 710ns per op measured). ScalarE gets the LAST-consumed score
# positions so PE tracks VectorE's just-in-time stream first while ScalarE
# works ahead. GpSimd is useless for bulk elementwise work (software
# TensorScalar measures ~7.5us per [128,512] op) -- it only dispatches DMAs.
ACT_RELU_KS = (6, 14, 7, 15)


def build_program():
    nc = bacc.Bacc(
        "TRN2", target_bir_lowering=False, debug=False, num_devices=NCORES
    )

    d_hT = nc.dram_tensor("hT", [D, N], f16, kind="ExternalInput").ap()
    d_WlT = nc.dram_tensor("WlT", [D, H * F], f16, kind="ExternalInput").ap()
    d_WrT = nc.dram_tensor("WrT", [D, H * F], f16, kind="ExternalInput").ap()
    # Amask[:, 56:64] is 0.8*A_aw (zeros elsewhere): 16 accumulating M=64
    # matmuls (8 per 64-row half, alternating PE column strips so LDWEIGHTS
    # overlaps the other half's in-flight matmul) compose 16 target nodes
    # into one full-height PSUM tile. lrelu = 0.8*relu + 0.2*identity; the
    # 0.8 is folded into these weights.
    d_Aaw = nc.dram_tensor("Amask", [H * F, 120], f16, kind="ExternalInput").ap()
    d_Aaw02 = nc.dram_tensor("Aaw02", [H * F, H], f16, kind="ExternalInput").ap()
    # Wlm [24,128]: rows 0-7 replicate the 0.2*alT linear term over i_local
    # (select h = p%8); rows 8-23 replicate 30*mask rows over h (select
    # il = p//8). One K=24 matmul seeds each group's score PSUM.
    d_Wlm = nc.dram_tensor("Wlm", [24, 128], f16, kind="ExternalInput").ap()
    # Rmask[:, 64q:64q+64] holds the head-sum reduction matrix placed in
    # columns [16q:16q+16] (4 accumulating matmuls -> one 64-row PSUM stripe).
    d_R = nc.dram_tensor("Rmask", [128, 4 * 64], f16, kind="ExternalInput").ap()
    d_hm = nc.dram_tensor("headmask", [128, H * F], f32, kind="ExternalInput").ap()
    # mask30[il, 512g + j] = 30*adj row for target node 16g+il: all 16
    # groups' mask blocks live in one [16, 16*512] tile, loaded ONCE at
    # setup (per-group DMAs would couple the transpose chain to the slow
    # SWDGE queue via framework DMA-ordering fences).
    d_mask = nc.dram_tensor("mask30", [GSIZE, NGROUPS * N], f16,
                            kind="ExternalInput").ap()
    d_out = nc.dram_tensor("out", [IHALF, D], f32, kind="ExternalOutput").ap()

    with tile.TileContext(nc) as tc:
        with ExitStack() as ctx:
            _gat_body(ctx, tc, d_out, d_hT, d_WlT, d_WrT, d_Aaw, d_Aaw02,
                      d_Wlm, d_R, d_hm, d_mask)
    nc.compile()
    return nc


def _gat_body(ctx, tc, d_out, d_hT, d_WlT, d_WrT, d_Aaw, d_Aaw02, d_Wlm,
              d_R, d_hm, d_mask):
    nc = tc.nc
    add = mybir.AluOpType.add
    mult = mybir.AluOpType.mult
    amax = mybir.AluOpType.max
    Relu = mybir.ActivationFunctionType.Relu
    Exp = mybir.ActivationFunctionType.Exp

    consts = ctx.enter_context(tc.tile_pool(name="consts", bufs=1))
    tpool = ctx.enter_context(tc.tile_pool(name="tpool", bufs=24))
    upool = ctx.enter_context(tc.tile_pool(name="upool", bufs=5))
    utpool = ctx.enter_context(tc.tile_pool(name="utpool", bufs=7))
    aggp = ctx.enter_context(tc.tile_pool(name="aggp", bufs=5))
    denp = ctx.enter_context(tc.tile_pool(name="denp", bufs=6))
    outp = ctx.enter_context(tc.tile_pool(name="outp", bufs=2))

    ppe = ctx.enter_context(tc.tile_pool(name="ppe", bufs=3, space="PSUM"))
    pagg = ctx.enter_context(tc.tile_pool(name="pagg", bufs=4, space="PSUM"))
    pout = ctx.enter_context(tc.tile_pool(name="pout", bufs=1, space="PSUM"))

    # ---- load constants (hT/W on the critical path first) ----
    s_hT = consts.tile([D, N], f16, tag="ht")
    nc.sync.dma_start(out=s_hT[:], in_=d_hT)
    s_WlT = consts.tile([D, H * F], f16, tag="wlt")
    nc.scalar.dma_start(out=s_WlT[:], in_=d_WlT)
    s_WrT = consts.tile([D, H * F], f16, tag="wrt")
    nc.scalar.dma_start(out=s_WrT[:], in_=d_WrT)
    s_Aaw = consts.tile([H * F, 120], f16, tag="aaw")
    nc.gpsimd.dma_start(out=s_Aaw[:], in_=d_Aaw)
    s_Aaw02 = consts.tile([H * F, H], f16, tag="aaw02")
    nc.gpsimd.dma_start(out=s_Aaw02[:], in_=d_Aaw02)
    s_Wlm = consts.tile([24, 128], f16, tag="wlm")
    nc.gpsimd.dma_start(out=s_Wlm[:], in_=d_Wlm)
    # rows 0-7: 0.2*alT replicated into every group block at setup (below);
    # rows 8-23: all 16 groups' 30*mask blocks, loaded once
    s_seed = consts.tile([24, NGROUPS * N], f16, tag="seed")
    nc.gpsimd.dma_start(out=s_seed[H:24, :], in_=d_mask)
    s_R = consts.tile([128, 4 * 64], f16, tag="rmat")
    nc.gpsimd.dma_start(out=s_R[:], in_=d_R)
    s_hm = consts.tile([128, H * F], f32, tag="hm")
    nc.gpsimd.dma_start(out=s_hm[:], in_=d_hm)
    neg30 = consts.tile([128, 1], f32, tag="neg30")
    nc.vector.memset(neg30[:], -MBIG)

    # ---- setup: projections ----
    # g_lT[hf, j] = sum_d WlT[d, hf] * hT[d, j]  (fp16: feeds the fp16
    # score path only)
    g_lT = consts.tile([H * F, N], f16, tag="glt")
    ps = ppe.tile([128, N], f32, tag="pe")
    nc.tensor.matmul(ps[:], s_WlT[:], s_hT[:], start=True, stop=True)
    nc.scalar.copy(g_lT[:], ps[:])

    g_rT = consts.tile([H * F, N], f32, tag="grt")
    ps = ppe.tile([128, N], f32, tag="pe")
    nc.tensor.matmul(ps[:], s_WrT[:], s_hT[:], start=True, stop=True)
    nc.vector.tensor_copy(g_rT[:], ps[:])

    # 0.2*alT[h, j] = sum_hf 0.2*Aaw[hf, h] * g_lT[hf, j]  -> fp16 [8, 512]
    alsb = consts.tile([H, N], f16, tag="alsb")
    psl = pagg.tile([H, N], f32, tag="agg")
    nc.tensor.matmul(psl[:], s_Aaw02[:], g_lT[:], start=True, stop=True)
    nc.vector.tensor_copy(alsb[:], psl[:])
    # replicate alT02 into rows 0-7 of every group's seed block (one DMA
    # with a zero-stride source block dimension)
    al_src = bass.AP(alsb.tensor, alsb[:].offset,
                     [list(alsb[:].ap[0]), [0, NGROUPS], [1, N]])
    al_dst = bass.AP(s_seed.tensor, s_seed[:].offset,
                     [[s_seed[:].ap[0][0], H], [N, NGROUPS], [1, N]])
    nc.gpsimd.dma_start(out=al_dst, in_=al_src)

    # g_r natural layout with an appended ones column per 128-row chunk:
    # chunk c occupies cols [129c, 129c+129); col 129c+128 is all-ones so the
    # aggregation matmul also produces the softmax denominator for free.
    g_r_nat = consts.tile([128, 4 * 129], f16, tag="grnat")
    for c in range(4):
        cs = slice(128 * c, 128 * (c + 1))
        pq = pagg.tile([128, 129], f32, tag="agg")
        nc.tensor.matmul(pq[:, 0:128], s_hT[:, cs], s_WrT[:],
                         start=True, stop=True)
        nc.vector.tensor_copy(g_r_nat[:, 129 * c:129 * c + 128], pq[:, 0:128])
        nc.vector.memset(g_r_nat[:, 129 * c + 128:129 * c + 129], 1.0)

    # ---- main loop: one group of 16 target nodes per iteration ----
    # Stage S(g): relus + score matmuls for group g (emitted in iteration g)
    # Stage E(g): exp + den + dma-transpose     (emitted in iteration g+1)
    # Stage P(g): agg, 1/den, normalize, output (emitted in iteration g+2)
    # visit halves alternately (0,8,1,9,...) so consecutive matmuls hit
    # different PE column strips and weight loads overlap compute
    order = [x for p_ in zip(range(8), range(8, 16)) for x in p_]
    state = {"out_ps": None}

    def escore(g):
        e_ps = ppe.tile([128, N], f32, tag="pe")
        # seed PSUM with lin term + mask boost (one K=24 matmul on this
        # group's block of the preloaded seed tile)
        nc.tensor.matmul(e_ps[:], s_Wlm[:], s_seed[:, N * g:N * (g + 1)],
                         start=True, stop=False, skip_group_check=True)
        for n_, k in enumerate(order):
            i = GSIZE * g + k
            t_t = tpool.tile([H * F, N], f16, tag="t")
            if k in ACT_RELU_KS:
                nc.scalar.activation(t_t[:], g_lT[:], Relu,
                                     bias=g_rT[:, i:i + 1], scale=1.0)
            else:
                # (g_lT + bias) max 0.0, one DVE pass (packed 16-bit mode)
                nc.vector.tensor_scalar(t_t[:], g_lT[:], g_rT[:, i:i + 1],
                                        0.0, add, amax)
            half = 64 * (k // 8)
            w0 = 56 - 8 * (k % 8)
            nc.tensor.matmul(e_ps[half:half + 64, :],
                             s_Aaw[:, w0:w0 + 64], t_t[:],
                             start=False, stop=(n_ >= len(order) - 2),
                             tile_position=(0, half),
                             skip_group_check=True)
        return e_ps

    def expt(e_ps):
        # u = exp(e - 30): unmasked j give exp(e) exactly; masked j
        # underflow fp16 to 0. accum_out yields den = sum_j u, so the
        # reciprocal depends only on ScalarE (never on the PE agg stream).
        u = upool.tile([128, N], f16, tag="u")
        den = denp.tile([128, 1], f32, tag="den")
        nc.scalar.activation(u[:], e_ps[:], Exp, bias=neg30[:],
                             accum_out=den[:])
        rd = denp.tile([128, 1], f32, tag="rden")
        nc.vector.reciprocal(rd[:], den[:])
        # transpose u via the DMA XBAR (one 16x128-tiled instruction):
        # uT[p, c, m] = u[m, 128c+p], so uT[:, c, :] is the aggregation
        # stationary for source-node chunk c.
        uT = utpool.tile([128, 4, 128], f16, tag="ut")
        nc.sync.dma_start_transpose(out=uT[:, :, :], in_=u[:])
        return (uT, rd)

    def post_pe(g, uT):
        # aggT[(il,h), hf] = sum_j u[(il,h), j] * g_r[j, hf]; the ones
        # column of g_r_nat yields den = sum_j u in col 128.
        agg_ps = pagg.tile([128, 129], f32, tag="agg")
        for c in range(4):
            nc.tensor.matmul(agg_ps[:], uT[:, c, :],
                             g_r_nat[:, 129 * c:129 * c + 129],
                             start=(c == 0), stop=(c == 3))
        return agg_ps

    def post_dve(g, agg_ps, rd):
        if g % 8 == 0:
            out_ps_t = pout.tile([128, D], f32, tag="out")
            state["out_ps"] = out_ps_t
        out_ps = state["out_ps"]

        # normalize rows by 1/den, keep only the matching head block
        agg_sb = aggp.tile([128, D], f16, tag="aggsb")
        nc.vector.scalar_tensor_tensor(
            agg_sb[:], agg_ps[:, 0:128], rd[:], s_hm[:], mult, mult)

        # out[16q + il, hf] = sum_h agg_sb[(il,h), hf]; 4 groups
        # accumulate into a 64-row stripe via zero-masked weights.
        q = g % 4
        stripe = 64 * ((g % 8) // 4)
        nc.tensor.matmul(out_ps[stripe:stripe + 64, :],
                         s_R[:, 64 * q:64 * q + 64], agg_sb[:],
                         start=(q == 0), stop=(q == 3))
        if q == 3:
            outb = outp.tile([64, D], f32, tag="outb")
            nc.scalar.copy(outb[:], out_ps[stripe:stripe + 64, :])
            r0 = 64 * (g // 4)
            nc.gpsimd.dma_start(out=d_out[r0:r0 + 64, :], in_=outb[:])

    # S(g) in iteration g, E(g) in g+1, agg(g) in g+4, norm/out(g) in g+5:
    # the exp -> dma-transpose -> agg chain costs ~5us (queue + xbar +
    # ~1.2us DMA semaphore propagation), so agg gets three group-periods
    # of slack, and the reciprocal/normalize always find their agg done
    # (no head-of-line blocking of the next group's relus on VectorE).
    eq = []   # [(g, e_ps)] awaiting E
    tq = []   # [(g, uT)] awaiting agg
    aq = []   # [(g, agg_ps)] awaiting norm/out
    for g in range(NGROUPS):
        e_ps = escore(g)
        if eq:
            ge, pe_ = eq.pop(0)
            tq.append((ge, *expt(pe_)))
        if len(tq) > 3:
            ga, uT, rd = tq.pop(0)
            aq.append((ga, post_pe(ga, uT), rd))
        if len(aq) > 2:
            post_dve(*aq.pop(0))
        eq.append((g, e_ps))

    ge, pe_ = eq.pop(0)
    tq.append((ge, *expt(pe_)))
    while tq or aq:
        if tq:
            ga, uT, rd = tq.pop(0)
            aq.append((ga, post_pe(ga, uT), rd))
        if aq:
            post_dve(*aq.pop(0))


def _host_inputs(h, adj, W_l, W_r, a_w):
    """Build the per-core input maps (pure layout/constant prep)."""
    HF = H * F
    Aaw = np.zeros((HF, H), dtype=np.float32)
    for hh in range(H):
        Aaw[hh * F:(hh + 1) * F, hh] = a_w
    Amask = np.zeros((HF, 120), dtype=np.float32)
    Amask[:, 56:64] = (1.0 - NEG_SLOPE) * Aaw
    Amask = Amask.astype(np.float16)
    Aaw02 = (NEG_SLOPE * Aaw).astype(np.float16)
    Wlm = np.zeros((24, 128), dtype=np.float16)
    for p in range(128):
        Wlm[p % H, p] = 1.0          # lin term: select h
        Wlm[H + p // H, p] = 1.0     # mask boost: select i_local
    Rmask = np.zeros((128, 4 * 64), dtype=np.float16)
    for q in range(4):
        for il in range(GSIZE):
            Rmask[il * H:(il + 1) * H, 64 * q + 16 * q + il] = 1.0
    headmask = np.zeros((128, HF), dtype=np.float32)
    for il in range(GSIZE):
        for hh in range(H):
            headmask[il * H + hh, hh * F:(hh + 1) * F] = 1.0
    WlT = np.ascontiguousarray(W_l.T).astype(np.float16)
    WrT = np.ascontiguousarray(W_r.T).astype(np.float16)

    in_maps = []
    for c in range(NCORES):
        b = c // 2
        i0 = IHALF * (c % 2)
        # Roll the node axis so this core's target nodes sit at positions
        # 0..IHALF-1 (the SPMD program indexes g_rT bias columns by local i).
        # Source-node order is permuted consistently everywhere (softmax and
        # aggregation are permutation-invariant over j).
        in_maps.append({
            "hT": np.ascontiguousarray(np.roll(h[b], -i0, axis=0).T).astype(
                np.float16),
            "WlT": WlT,
            "WrT": WrT,
            "Amask": Amask,
            "Aaw02": Aaw02,
            "Wlm": Wlm,
            "Rmask": Rmask,
            "headmask": headmask,
            "mask30": np.ascontiguousarray(
                (MBIG * np.roll(adj[b, i0:i0 + IHALF, :, 0], -i0, axis=1))
                .reshape(NGROUPS, GSIZE, N).transpose(1, 0, 2)
                .reshape(GSIZE, NGROUPS * N)).astype(np.float16),
        })
    return in_maps


_NC_CACHE = {}
LAST_RESULT = None  # BassKernelResults of the most recent kernel() call


def _get_program():
    if "nc" not in _NC_CACHE:
        _NC_CACHE["nc"] = build_program()
    return _NC_CACHE["nc"]


def kernel(h, adj, W_l, W_r, a_w):
    h = np.asarray(h)
    adj = np.asarray(adj)
    W_l = np.asarray(W_l)
    W_r = np.asarray(W_r)
    a_w = np.asarray(a_w)

    nc = _get_program()
    in_maps = _host_inputs(h, adj, W_l, W_r, a_w)
    res = None
    for attempt in range(3):
        try:
            res = run_bass_kernel_spmd(nc, in_maps, list(range(NCORES)))
            break
        except Exception:
            # the axon-proxied device occasionally reports a transient
            # "unrecoverable" state at process start; it self-heals
            if attempt == 2:
                raise
            import time
            time.sleep(20)
    global LAST_RESULT
    LAST_RESULT = res

    out = np.zeros((B, N, D), dtype=np.float32)
    for c in range(NCORES):
        b = c // 2
        i0 = IHALF * (c % 2)
        out[b, i0:i0 + IHALF, :] = res.results[c]["out"]
    return out


# revision 28
# speedup vs baseline: 1.2083x; 1.1024x over previous
"""GATv2 layer kernel for Trainium2 (Bass/Tile), 8-core SPMD.

Problem (hardcoded): B=4, N=512, D=128, H=8 heads, F=16 hidden, is_concat.
  g_l = h @ W_l.T ; g_r = h @ W_r.T               [B,N,H,F]
  e[b,i,j,h] = sum_f a_w[f]*lrelu(g_l[b,j,h,f] + g_r[b,i,h,f], 0.2)
  a = softmax_j(e masked by adj)                  [B,i,j,H]
  out[b,i,h,f] = sum_j a[b,i,j,h]*g_r[b,j,h,f]   -> [B,N,H*F]

Sharding: 8 cores = (batch b in 0..4) x (i-half in 0..2). Each core handles
256 target nodes i of one batch with fully-replicated g_l/g_r.

Math used on device (per core, b fixed):
  lrelu(x) = 0.8*relu(x) + 0.2*x, so
  e[i,j,h] = 0.8*sum_hf A[hf,h]*relu(g_lT[hf,j] + g_rT[hf,i]) + 0.2*alT[h,j]
             + 0.2*ar[i,h]
  The per-(i,h) additive term 0.2*ar cancels in softmax (shift invariance),
  so it is dropped. The adjacency mask is folded into the scores as an
  additive +30*mask accumulated by the same PSUM matmul that adds the
  0.2*alT linear term (K=24 stationary selector); exp then runs with bias
  -30: unmasked j recover exp(e) exactly, masked j give exp(e-30) which
  underflows fp16 to exact 0. Softmax normalization is applied after
  aggregation (linearity), with the denominator produced for free by an
  all-ones column appended to g_r.

Per group of 16 i's (PSUM partitions = (i_local*8 + h)):
  M[24,512]     = [0.2*alT (8 rows); 30*mask rows (16)]   DMA-filled
  psum          = Wlm.T @ M  (start)                      PE  (K=24)
  t[hf, j]      = relu(g_lT + g_rT[:, i] bias)            DVE/ACT/GPSIMD
  psum += A_aw.T @ t  (M=64 strips, alternating halves)   PE
  u, den        = exp(psum - 30), rowsum via accum_out    ACT (reads PSUM)
  uT            = dma transpose (16x128 XBAR tiles)       DMA engines
  agg_ps        = sum_j uT.T @ g_r_nat (4 K-chunks)       PE
  rd            = 1/den                                   DVE
  agg_sb        = agg_ps * rd * headmask                  DVE (fused)
  out[i,hf]     = R.T @ agg_sb  (sum over h)              PE
The exp/transpose of group g and the aggregation of group g-1 are emitted
one/two iterations late (software pipelining): each engine's queue is
in-order, so this keeps PE streaming scores and ACT streaming relus with
no head-of-line blocking on cross-engine dependencies.
"""

import ml_dtypes
import numpy as np
from contextlib import ExitStack

import concourse.bass as bass
import concourse.bacc as bacc
import concourse.tile as tile
import concourse.mybir as mybir
from concourse.bass_utils import run_bass_kernel_spmd

B, N, D = 4, 512, 128
H, F = 8, 16
NEG_SLOPE = 0.2
NCORES = 8
IHALF = N // 2          # 256 target nodes per core
GSIZE = 16              # i's per group
NGROUPS = IHALF // GSIZE  # 16
MBIG = 30.0             # mask boost; exp(e-30) underflows fp16 to exact 0
f32 = mybir.dt.float32
f16 = mybir.dt.float16

# The score path (relu'd pairwise features t and the per-node score matmuls)
# runs in fp16: full 1-cycle/row PE streaming and DVE packed 16-bit modes --
# with a 10-bit mantissa and ample range for these tiny values. The noise
# lands only in pre-softmax scores; softmax normalization and the
# aggregation path stay exact fp32.

# Which of the 16 per-group relu ops run on ScalarE (the rest on VectorE:
# 347ns vs 710ns per op measured). ScalarE gets the LAST-consumed score
# positions so PE tracks VectorE's just-in-time stream first while ScalarE
# works ahead. GpSimd is useless for bulk elementwise work (software
# TensorScalar measures ~7.5us per [128,512] op) -- it only dispatches DMAs.
ACT_RELU_KS = (6, 14, 7, 15)


def build_program():
    nc = bacc.Bacc(
        "TRN2", target_bir_lowering=False, debug=False, num_devices=NCORES
    )

    d_hT = nc.dram_tensor("hT", [D, N], f16, kind="ExternalInput").ap()
    d_WlT = nc.dram_tensor("WlT", [D, H * F], f16, kind="ExternalInput").ap()
    d_WrT = nc.dram_tensor("WrT", [D, H * F], f16, kind="ExternalInput").ap()
    # Amask[:, 56:64] is 0.8*A_aw (zeros elsewhere): 16 accumulating M=64
    # matmuls (8 per 64-row half, alternating PE column strips so LDWEIGHTS
    # overlaps the other half's in-flight matmul) compose 16 target nodes
    # into one full-height PSUM tile. lrelu = 0.8*relu + 0.2*identity; the
    # 0.8 is folded into these weights.
    d_Aaw = nc.dram_tensor("Amask", [H * F, 120], f16, kind="ExternalInput").ap()
    d_Aaw02 = nc.dram_tensor("Aaw02", [H * F, H], f16, kind="ExternalInput").ap()
    # Wlm [24,128]: rows 0-7 replicate the 0.2*alT linear term over i_local
    # (select h = p%8); rows 8-23 replicate 30*mask rows over h (select
    # il = p//8). One K=24 matmul seeds each group's score PSUM.
    d_Wlm = nc.dram_tensor("Wlm", [24, 128], f16, kind="ExternalInput").ap()
    # Rmask[:, 64q:64q+64] holds the head-sum reduction matrix placed in
    # columns [16q:16q+16] (4 accumulating matmuls -> one 64-row PSUM stripe).
    d_R = nc.dram_tensor("Rmask", [128, 4 * 64], f16, kind="ExternalInput").ap()
    d_hm = nc.dram_tensor("headmask", [128, H * F], f32, kind="ExternalInput").ap()
    # mask30[il, 512g + j] = 30*adj row for target node 16g+il: all 16
    # groups' mask blocks live in one [16, 16*512] tile, loaded ONCE at
    # setup (per-group DMAs would couple the transpose chain to the slow
    # SWDGE queue via framework DMA-ordering fences).
    d_mask = nc.dram_tensor("mask30", [GSIZE, NGROUPS * N], f16,
                            kind="ExternalInput").ap()
    d_out = nc.dram_tensor("out", [IHALF, D], f32, kind="ExternalOutput").ap()

    with tile.TileContext(nc) as tc:
        with ExitStack() as ctx:
            _gat_body(ctx, tc, d_out, d_hT, d_WlT, d_WrT, d_Aaw, d_Aaw02,
                      d_Wlm, d_R, d_hm, d_mask)
    nc.compile()
    return nc


def _gat_body(ctx, tc, d_out, d_hT, d_WlT, d_WrT, d_Aaw, d_Aaw02, d_Wlm,
              d_R, d_hm, d_mask):
    nc = tc.nc
    add = mybir.AluOpType.add
    mult = mybir.AluOpType.mult
    amax = mybir.AluOpType.max
    Relu = mybir.ActivationFunctionType.Relu
    Exp = mybir.ActivationFunctionType.Exp

    consts = ctx.enter_context(tc.tile_pool(name="consts", bufs=1))
    tpool = ctx.enter_context(tc.tile_pool(name="tpool", bufs=24))
    upool = ctx.enter_context(tc.tile_pool(name="upool", bufs=5))
    utpool = ctx.enter_context(tc.tile_pool(name="utpool", bufs=7))
    aggp = ctx.enter_context(tc.tile_pool(name="aggp", bufs=5))
    denp = ctx.enter_context(tc.tile_pool(name="denp", bufs=6))
    outp = ctx.enter_context(tc.tile_pool(name="outp", bufs=2))

    ppe = ctx.enter_context(tc.tile_pool(name="ppe", bufs=3, space="PSUM"))
    pagg = ctx.enter_context(tc.tile_pool(name="pagg", bufs=4, space="PSUM"))
    pout = ctx.enter_context(tc.tile_pool(name="pout", bufs=1, space="PSUM"))

    # ---- load constants (hT/W on the critical path first) ----
    s_hT = consts.tile([D, N], f16, tag="ht")
    nc.sync.dma_start(out=s_hT[:], in_=d_hT)
    s_WlT = consts.tile([D, H * F], f16, tag="wlt")
    nc.scalar.dma_start(out=s_WlT[:], in_=d_WlT)
    s_WrT = consts.tile([D, H * F], f16, tag="wrt")
    nc.scalar.dma_start(out=s_WrT[:], in_=d_WrT)
    s_Aaw = consts.tile([H * F, 120], f16, tag="aaw")
    nc.gpsimd.dma_start(out=s_Aaw[:], in_=d_Aaw)
    s_Aaw02 = consts.tile([H * F, H], f16, tag="aaw02")
    nc.gpsimd.dma_start(out=s_Aaw02[:], in_=d_Aaw02)
    s_Wlm = consts.tile([24, 128], f16, tag="wlm")
    nc.gpsimd.dma_start(out=s_Wlm[:], in_=d_Wlm)
    # rows 0-7: 0.2*alT replicated into every group block at setup (below);
    # rows 8-23: all 16 groups' 30*mask blocks, loaded once
    s_seed = consts.tile([24, NGROUPS * N], f16, tag="seed")
    nc.gpsimd.dma_start(out=s_seed[H:24, :], in_=d_mask)
    s_R = consts.tile([128, 4 * 64], f16, tag="rmat")
    nc.gpsimd.dma_start(out=s_R[:], in_=d_R)
    s_hm = consts.tile([128, H * F], f32, tag="hm")
    nc.gpsimd.dma_start(out=s_hm[:], in_=d_hm)
    neg30 = consts.tile([128, 1], f32, tag="neg30")
    nc.vector.memset(neg30[:], -MBIG)

    # ---- setup: projections ----
    # g_lT[hf, j] = sum_d WlT[d, hf] * hT[d, j]  (fp16: feeds the fp16
    # score path only)
    g_lT = consts.tile([H * F, N], f16, tag="glt")
    ps = ppe.tile([128, N], f32, tag="pe")
    nc.tensor.matmul(ps[:], s_WlT[:], s_hT[:], start=True, stop=True)
    nc.scalar.copy(g_lT[:], ps[:])

    g_rT = consts.tile([H * F, N], f32, tag="grt")
    ps = ppe.tile([128, N], f32, tag="pe")
    nc.tensor.matmul(ps[:], s_WrT[:], s_hT[:], start=True, stop=True)
    nc.scalar.copy(g_rT[:], ps[:])

    # 0.2*alT[h, j] = sum_hf 0.2*Aaw[hf, h] * g_lT[hf, j]  -> fp16 [8, 512]
    alsb = consts.tile([H, N], f16, tag="alsb")
    psl = pagg.tile([H, N], f32, tag="agg")
    nc.tensor.matmul(psl[:], s_Aaw02[:], g_lT[:], start=True, stop=True)
    nc.scalar.copy(alsb[:], psl[:])
    # replicate alT02 into rows 0-7 of every group's seed block (one DMA
    # with a zero-stride source block dimension)
    al_src = bass.AP(alsb.tensor, alsb[:].offset,
                     [list(alsb[:].ap[0]), [0, NGROUPS], [1, N]])
    al_dst = bass.AP(s_seed.tensor, s_seed[:].offset,
                     [[s_seed[:].ap[0][0], H], [N, NGROUPS], [1, N]])
    nc.sync.dma_start(out=al_dst, in_=al_src)

    # g_r natural layout with an appended ones column per 128-row chunk:
    # chunk c occupies cols [129c, 129c+129); col 129c+128 is all-ones so the
    # aggregation matmul also produces the softmax denominator for free.
    g_r_nat = consts.tile([128, 4 * 129], f16, tag="grnat")
    for c in range(4):
        cs = slice(128 * c, 128 * (c + 1))
        pq = pagg.tile([128, 129], f32, tag="agg")
        nc.tensor.matmul(pq[:, 0:128], s_hT[:, cs], s_WrT[:],
                         start=True, stop=True)
        nc.scalar.copy(g_r_nat[:, 129 * c:129 * c + 128], pq[:, 0:128])
        nc.vector.memset(g_r_nat[:, 129 * c + 128:129 * c + 129], 1.0)

    # ---- main loop: one group of 16 target nodes per iteration ----
    # Stage S(g): relus + score matmuls for group g (emitted in iteration g)
    # Stage E(g): exp + den + dma-transpose     (emitted in iteration g+1)
    # Stage P(g): agg, 1/den, normalize, output (emitted in iteration g+2)
    # visit halves alternately (0,8,1,9,...) so consecutive matmuls hit
    # different PE column strips and weight loads overlap compute
    order = [x for p_ in zip(range(8), range(8, 16)) for x in p_]
    state = {"out_ps": None}

    def escore(g):
        e_ps = ppe.tile([128, N], f32, tag="pe")
        # seed PSUM with lin term + mask boost (one K=24 matmul on this
        # group's block of the preloaded seed tile)
        nc.tensor.matmul(e_ps[:], s_Wlm[:], s_seed[:, N * g:N * (g + 1)],
                         start=True, stop=False, skip_group_check=True)
        for n_, k in enumerate(order):
            i = GSIZE * g + k
            t_t = tpool.tile([H * F, N], f16, tag="t")
            if k in ACT_RELU_KS:
                nc.scalar.activation(t_t[:], g_lT[:], Relu,
                                     bias=g_rT[:, i:i + 1], scale=1.0)
            else:
                # (g_lT + bias) max 0.0, one DVE pass (packed 16-bit mode)
                nc.vector.tensor_scalar(t_t[:], g_lT[:], g_rT[:, i:i + 1],
                                        0.0, add, amax)
            half = 64 * (k // 8)
            w0 = 56 - 8 * (k % 8)
            nc.tensor.matmul(e_ps[half:half + 64, :],
                             s_Aaw[:, w0:w0 + 64], t_t[:],
                             start=False, stop=(n_ >= len(order) - 2),
                             tile_position=(0, half),
                             skip_group_check=True)
        return e_ps

    def expt(e_ps):
        # u = exp(e - 30): unmasked j give exp(e) exactly; masked j
        # underflow fp16 to 0. accum_out yields den = sum_j u, so the
        # reciprocal depends only on ScalarE (never on the PE agg stream).
        u = upool.tile([128, N], f16, tag="u")
        den = denp.tile([128, 1], f32, tag="den")
        nc.scalar.activation(u[:], e_ps[:], Exp, bias=neg30[:],
                             accum_out=den[:])
        rd = denp.tile([128, 1], f32, tag="rden")
        nc.vector.reciprocal(rd[:], den[:])
        # transpose u via the DMA XBAR (one 16x128-tiled instruction):
        # uT[p, c, m] = u[m, 128c+p], so uT[:, c, :] is the aggregation
        # stationary for source-node chunk c.
        uT = utpool.tile([128, 4, 128], f16, tag="ut")
        nc.sync.dma_start_transpose(out=uT[:, :, :], in_=u[:])
        return (uT, rd)

    def post_pe(g, uT):
        # aggT[(il,h), hf] = sum_j u[(il,h), j] * g_r[j, hf]; the ones
        # column of g_r_nat yields den = sum_j u in col 128.
        agg_ps = pagg.tile([128, 129], f32, tag="agg")
        for c in range(4):
            nc.tensor.matmul(agg_ps[:], uT[:, c, :],
                             g_r_nat[:, 129 * c:129 * c + 129],
                             start=(c == 0), stop=(c == 3))
        return agg_ps

    def post_dve(g, agg_ps, rd):
        if g % 8 == 0:
            out_ps_t = pout.tile([128, D], f32, tag="out")
            state["out_ps"] = out_ps_t
        out_ps = state["out_ps"]

        # normalize rows by 1/den, keep only the matching head block
        agg_sb = aggp.tile([128, D], f16, tag="aggsb")
        nc.vector.scalar_tensor_tensor(
            agg_sb[:], agg_ps[:, 0:128], rd[:], s_hm[:], mult, mult)

        # out[16q + il, hf] = sum_h agg_sb[(il,h), hf]; 4 groups
        # accumulate into a 64-row stripe via zero-masked weights.
        q = g % 4
        stripe = 64 * ((g % 8) // 4)
        nc.tensor.matmul(out_ps[stripe:stripe + 64, :],
                         s_R[:, 64 * q:64 * q + 64], agg_sb[:],
                         start=(q == 0), stop=(q == 3))
        if q == 3:
            outb = outp.tile([64, D], f32, tag="outb")
            nc.scalar.copy(outb[:], out_ps[stripe:stripe + 64, :])
            r0 = 64 * (g // 4)
            nc.gpsimd.dma_start(out=d_out[r0:r0 + 64, :], in_=outb[:])

    # S(g) in iteration g, E(g) in g+1, agg(g) in g+4, norm/out(g) in g+5:
    # the exp -> dma-transpose -> agg chain costs ~5us (queue + xbar +
    # ~1.2us DMA semaphore propagation), so agg gets three group-periods
    # of slack, and the reciprocal/normalize always find their agg done
    # (no head-of-line blocking of the next group's relus on VectorE).
    eq = []   # [(g, e_ps)] awaiting E
    tq = []   # [(g, uT)] awaiting agg
    aq = []   # [(g, agg_ps)] awaiting norm/out
    for g in range(NGROUPS):
        e_ps = escore(g)
        if eq:
            ge, pe_ = eq.pop(0)
            tq.append((ge, *expt(pe_)))
        npe = 1 if g < 13 else 2
        ndve = 1 if g < 13 else 2
        while npe > 0 and len(tq) > (3 if g < 13 else 1):
            ga, uT, rd = tq.pop(0)
            aq.append((ga, post_pe(ga, uT), rd))
            npe -= 1
        while ndve > 0 and len(aq) > (2 if g < 13 else 1):
            post_dve(*aq.pop(0))
            ndve -= 1
        eq.append((g, e_ps))

    ge, pe_ = eq.pop(0)
    tq.append((ge, *expt(pe_)))
    while tq or aq:
        if tq:
            ga, uT, rd = tq.pop(0)
            aq.append((ga, post_pe(ga, uT), rd))
        if aq:
            post_dve(*aq.pop(0))


def _host_inputs(h, adj, W_l, W_r, a_w):
    """Build the per-core input maps (pure layout/constant prep)."""
    HF = H * F
    Aaw = np.zeros((HF, H), dtype=np.float32)
    for hh in range(H):
        Aaw[hh * F:(hh + 1) * F, hh] = a_w
    Amask = np.zeros((HF, 120), dtype=np.float32)
    Amask[:, 56:64] = (1.0 - NEG_SLOPE) * Aaw
    Amask = Amask.astype(np.float16)
    Aaw02 = (NEG_SLOPE * Aaw).astype(np.float16)
    Wlm = np.zeros((24, 128), dtype=np.float16)
    for p in range(128):
        Wlm[p % H, p] = 1.0          # lin term: select h
        Wlm[H + p // H, p] = 1.0     # mask boost: select i_local
    Rmask = np.zeros((128, 4 * 64), dtype=np.float16)
    for q in range(4):
        for il in range(GSIZE):
            Rmask[il * H:(il + 1) * H, 64 * q + 16 * q + il] = 1.0
    headmask = np.zeros((128, HF), dtype=np.float32)
    for il in range(GSIZE):
        for hh in range(H):
            headmask[il * H + hh, hh * F:(hh + 1) * F] = 1.0
    WlT = np.ascontiguousarray(W_l.T).astype(np.float16)
    WrT = np.ascontiguousarray(W_r.T).astype(np.float16)

    in_maps = []
    for c in range(NCORES):
        b = c // 2
        i0 = IHALF * (c % 2)
        # Roll the node axis so this core's target nodes sit at positions
        # 0..IHALF-1 (the SPMD program indexes g_rT bias columns by local i).
        # Source-node order is permuted consistently everywhere (softmax and
        # aggregation are permutation-invariant over j).
        in_maps.append({
            "hT": np.ascontiguousarray(np.roll(h[b], -i0, axis=0).T).astype(
                np.float16),
            "WlT": WlT,
            "WrT": WrT,
            "Amask": Amask,
            "Aaw02": Aaw02,
            "Wlm": Wlm,
            "Rmask": Rmask,
            "headmask": headmask,
            "mask30": np.ascontiguousarray(
                (MBIG * np.roll(adj[b, i0:i0 + IHALF, :, 0], -i0, axis=1))
                .reshape(NGROUPS, GSIZE, N).transpose(1, 0, 2)
                .reshape(GSIZE, NGROUPS * N)).astype(np.float16),
        })
    return in_maps


_NC_CACHE = {}
LAST_RESULT = None  # BassKernelResults of the most recent kernel() call


def _get_program():
    if "nc" not in _NC_CACHE:
        _NC_CACHE["nc"] = build_program()
    return _NC_CACHE["nc"]


def kernel(h, adj, W_l, W_r, a_w):
    h = np.asarray(h)
    adj = np.asarray(adj)
    W_l = np.asarray(W_l)
    W_r = np.asarray(W_r)
    a_w = np.asarray(a_w)

    nc = _get_program()
    in_maps = _host_inputs(h, adj, W_l, W_r, a_w)
    res = None
    for attempt in range(3):
        try:
            res = run_bass_kernel_spmd(nc, in_maps, list(range(NCORES)))
            break
        except Exception:
            # the axon-proxied device occasionally reports a transient
            # "unrecoverable" state at process start; it self-heals
            if attempt == 2:
                raise
            import time
            time.sleep(20)
    global LAST_RESULT
    LAST_RESULT = res

    out = np.zeros((B, N, D), dtype=np.float32)
    for c in range(NCORES):
        b = c // 2
        i0 = IHALF * (c % 2)
        out[b, i0:i0 + IHALF, :] = res.results[c]["out"]
    return out


# revision 29
# speedup vs baseline: 1.2088x; 1.0005x over previous
"""GATv2 layer kernel for Trainium2 (Bass/Tile), 8-core SPMD.

Problem (hardcoded): B=4, N=512, D=128, H=8 heads, F=16 hidden, is_concat.
  g_l = h @ W_l.T ; g_r = h @ W_r.T               [B,N,H,F]
  e[b,i,j,h] = sum_f a_w[f]*lrelu(g_l[b,j,h,f] + g_r[b,i,h,f], 0.2)
  a = softmax_j(e masked by adj)                  [B,i,j,H]
  out[b,i,h,f] = sum_j a[b,i,j,h]*g_r[b,j,h,f]   -> [B,N,H*F]

Sharding: 8 cores = (batch b in 0..4) x (i-half in 0..2). Each core handles
256 target nodes i of one batch with fully-replicated g_l/g_r.

Math used on device (per core, b fixed):
  lrelu(x) = 0.8*relu(x) + 0.2*x, so
  e[i,j,h] = 0.8*sum_hf A[hf,h]*relu(g_lT[hf,j] + g_rT[hf,i]) + 0.2*alT[h,j]
             + 0.2*ar[i,h]
  The per-(i,h) additive term 0.2*ar cancels in softmax (shift invariance),
  so it is dropped. The adjacency mask is folded into the scores as an
  additive +30*mask accumulated by the same PSUM matmul that adds the
  0.2*alT linear term (K=24 stationary selector); exp then runs with bias
  -30: unmasked j recover exp(e) exactly, masked j give exp(e-30) which
  underflows fp16 to exact 0. Softmax normalization is applied after
  aggregation (linearity), with the denominator produced for free by an
  all-ones column appended to g_r.

Per group of 16 i's (PSUM partitions = (i_local*8 + h)):
  M[24,512]     = [0.2*alT (8 rows); 30*mask rows (16)]   DMA-filled
  psum          = Wlm.T @ M  (start)                      PE  (K=24)
  t[hf, j]      = relu(g_lT + g_rT[:, i] bias)            DVE/ACT/GPSIMD
  psum += A_aw.T @ t  (M=64 strips, alternating halves)   PE
  u, den        = exp(psum - 30), rowsum via accum_out    ACT (reads PSUM)
  uT            = dma transpose (16x128 XBAR tiles)       DMA engines
  agg_ps        = sum_j uT.T @ g_r_nat (4 K-chunks)       PE
  rd            = 1/den                                   DVE
  agg_sb        = agg_ps * rd * headmask                  DVE (fused)
  out[i,hf]     = R.T @ agg_sb  (sum over h)              PE
The exp/transpose of group g and the aggregation of group g-1 are emitted
one/two iterations late (software pipelining): each engine's queue is
in-order, so this keeps PE streaming scores and ACT streaming relus with
no head-of-line blocking on cross-engine dependencies.
"""

import ml_dtypes
import numpy as np
from contextlib import ExitStack

import concourse.bass as bass
import concourse.bacc as bacc
import concourse.tile as tile
import concourse.mybir as mybir
from concourse.bass_utils import run_bass_kernel_spmd

B, N, D = 4, 512, 128
H, F = 8, 16
NEG_SLOPE = 0.2
NCORES = 8
IHALF = N // 2          # 256 target nodes per core
GSIZE = 16              # i's per group
NGROUPS = IHALF // GSIZE  # 16
MBIG = 30.0             # mask boost; exp(e-30) underflows fp16 to exact 0
f32 = mybir.dt.float32
f16 = mybir.dt.float16

# The score path (relu'd pairwise features t and the per-node score matmuls)
# runs in fp16: full 1-cycle/row PE streaming and DVE packed 16-bit modes --
# with a 10-bit mantissa and ample range for these tiny values. The noise
# lands only in pre-softmax scores; softmax normalization and the
# aggregation path stay exact fp32.

# Which of the 16 per-group relu ops run on ScalarE (the rest on VectorE:
# 347ns vs 710ns per op measured). ScalarE gets the LAST-consumed score
# positions so PE tracks VectorE's just-in-time stream first while ScalarE
# works ahead. GpSimd is useless for bulk elementwise work (software
# TensorScalar measures ~7.5us per [128,512] op) -- it only dispatches DMAs.
ACT_RELU_KS = (6, 14, 7, 15)


def build_program():
    nc = bacc.Bacc(
        "TRN2", target_bir_lowering=False, debug=False, num_devices=NCORES
    )

    d_hT = nc.dram_tensor("hT", [D, N], f16, kind="ExternalInput").ap()
    d_WlT = nc.dram_tensor("WlT", [D, H * F], f16, kind="ExternalInput").ap()
    d_WrT = nc.dram_tensor("WrT", [D, H * F], f16, kind="ExternalInput").ap()
    # Amask[:, 56:64] is 0.8*A_aw (zeros elsewhere): 16 accumulating M=64
    # matmuls (8 per 64-row half, alternating PE column strips so LDWEIGHTS
    # overlaps the other half's in-flight matmul) compose 16 target nodes
    # into one full-height PSUM tile. lrelu = 0.8*relu + 0.2*identity; the
    # 0.8 is folded into these weights.
    d_Aaw = nc.dram_tensor("Amask", [H * F, 120], f16, kind="ExternalInput").ap()
    d_Aaw02 = nc.dram_tensor("Aaw02", [H * F, H], f16, kind="ExternalInput").ap()
    # Wlm [24,128]: rows 0-7 replicate the 0.2*alT linear term over i_local
    # (select h = p%8); rows 8-23 replicate 30*mask rows over h (select
    # il = p//8). One K=24 matmul seeds each group's score PSUM.
    d_Wlm = nc.dram_tensor("Wlm", [24, 128], f16, kind="ExternalInput").ap()
    # Rmask[:, 64q:64q+64] holds the head-sum reduction matrix placed in
    # columns [16q:16q+16] (4 accumulating matmuls -> one 64-row PSUM stripe).
    d_R = nc.dram_tensor("Rmask", [128, 4 * 64], f16, kind="ExternalInput").ap()
    d_hm = nc.dram_tensor("headmask", [128, H * F], f32, kind="ExternalInput").ap()
    # mask30[il, 512g + j] = 30*adj row for target node 16g+il: all 16
    # groups' mask blocks live in one [16, 16*512] tile, loaded ONCE at
    # setup (per-group DMAs would couple the transpose chain to the slow
    # SWDGE queue via framework DMA-ordering fences).
    d_mask = nc.dram_tensor("mask30", [GSIZE, NGROUPS * N], f16,
                            kind="ExternalInput").ap()
    d_out = nc.dram_tensor("out", [IHALF, D], f32, kind="ExternalOutput").ap()

    with tile.TileContext(nc) as tc:
        with ExitStack() as ctx:
            _gat_body(ctx, tc, d_out, d_hT, d_WlT, d_WrT, d_Aaw, d_Aaw02,
                      d_Wlm, d_R, d_hm, d_mask)
    nc.compile()
    return nc


def _gat_body(ctx, tc, d_out, d_hT, d_WlT, d_WrT, d_Aaw, d_Aaw02, d_Wlm,
              d_R, d_hm, d_mask):
    nc = tc.nc
    add = mybir.AluOpType.add
    mult = mybir.AluOpType.mult
    amax = mybir.AluOpType.max
    Relu = mybir.ActivationFunctionType.Relu
    Exp = mybir.ActivationFunctionType.Exp

    consts = ctx.enter_context(tc.tile_pool(name="consts", bufs=1))
    tpool = ctx.enter_context(tc.tile_pool(name="tpool", bufs=24))
    upool = ctx.enter_context(tc.tile_pool(name="upool", bufs=5))
    utpool = ctx.enter_context(tc.tile_pool(name="utpool", bufs=7))
    aggp = ctx.enter_context(tc.tile_pool(name="aggp", bufs=5))
    denp = ctx.enter_context(tc.tile_pool(name="denp", bufs=6))
    outp = ctx.enter_context(tc.tile_pool(name="outp", bufs=2))

    ppe = ctx.enter_context(tc.tile_pool(name="ppe", bufs=3, space="PSUM"))
    pagg = ctx.enter_context(tc.tile_pool(name="pagg", bufs=4, space="PSUM"))
    pout = ctx.enter_context(tc.tile_pool(name="pout", bufs=1, space="PSUM"))

    # ---- load constants (hT/W on the critical path first) ----
    s_hT = consts.tile([D, N], f16, tag="ht")
    nc.sync.dma_start(out=s_hT[:], in_=d_hT)
    s_WlT = consts.tile([D, H * F], f16, tag="wlt")
    nc.scalar.dma_start(out=s_WlT[:], in_=d_WlT)
    s_WrT = consts.tile([D, H * F], f16, tag="wrt")
    nc.scalar.dma_start(out=s_WrT[:], in_=d_WrT)
    s_Aaw = consts.tile([H * F, 120], f16, tag="aaw")
    nc.gpsimd.dma_start(out=s_Aaw[:], in_=d_Aaw)
    s_Aaw02 = consts.tile([H * F, H], f16, tag="aaw02")
    nc.gpsimd.dma_start(out=s_Aaw02[:], in_=d_Aaw02)
    s_Wlm = consts.tile([24, 128], f16, tag="wlm")
    nc.gpsimd.dma_start(out=s_Wlm[:], in_=d_Wlm)
    # rows 0-7: 0.2*alT replicated into every group block at setup (below);
    # rows 8-23: all 16 groups' 30*mask blocks, loaded once
    s_seed = consts.tile([24, NGROUPS * N], f16, tag="seed")
    nc.gpsimd.dma_start(out=s_seed[H:24, :], in_=d_mask)
    s_R = consts.tile([128, 4 * 64], f16, tag="rmat")
    nc.gpsimd.dma_start(out=s_R[:], in_=d_R)
    s_hm = consts.tile([128, H * F], f32, tag="hm")
    nc.gpsimd.dma_start(out=s_hm[:], in_=d_hm)
    neg30 = consts.tile([128, 1], f32, tag="neg30")
    nc.vector.memset(neg30[:], -MBIG)

    # ---- setup: projections ----
    # g_lT[hf, j] = sum_d WlT[d, hf] * hT[d, j]  (fp16: feeds the fp16
    # score path only)
    g_lT = consts.tile([H * F, N], f16, tag="glt")
    ps = ppe.tile([128, N], f32, tag="pe")
    nc.tensor.matmul(ps[:], s_WlT[:], s_hT[:], start=True, stop=True)
    nc.scalar.copy(g_lT[:], ps[:])

    g_rT = consts.tile([H * F, N], f32, tag="grt")
    ps = ppe.tile([128, N], f32, tag="pe")
    nc.tensor.matmul(ps[:, 0:N // 2], s_WrT[:], s_hT[:, 0:N // 2],
                     start=True, stop=True)
    nc.tensor.matmul(ps[:, N // 2:N], s_WrT[:], s_hT[:, N // 2:N],
                     start=True, stop=True, skip_group_check=True)
    nc.scalar.copy(g_rT[:, 0:N // 2], ps[:, 0:N // 2])
    nc.scalar.copy(g_rT[:, N // 2:N], ps[:, N // 2:N])

    # 0.2*alT[h, j] = sum_hf 0.2*Aaw[hf, h] * g_lT[hf, j]  -> fp16 [8, 512]
    alsb = consts.tile([H, N], f16, tag="alsb")
    psl = pagg.tile([H, N], f32, tag="agg")
    nc.tensor.matmul(psl[:], s_Aaw02[:], g_lT[:], start=True, stop=True)
    nc.scalar.copy(alsb[:], psl[:])
    # replicate alT02 into rows 0-7 of every group's seed block (one DMA
    # with a zero-stride source block dimension)
    al_src = bass.AP(alsb.tensor, alsb[:].offset,
                     [list(alsb[:].ap[0]), [0, NGROUPS], [1, N]])
    al_dst = bass.AP(s_seed.tensor, s_seed[:].offset,
                     [[s_seed[:].ap[0][0], H], [N, NGROUPS], [1, N]])
    nc.sync.dma_start(out=al_dst, in_=al_src)

    # g_r natural layout with an appended ones column per 128-row chunk:
    # chunk c occupies cols [129c, 129c+129); col 129c+128 is all-ones so the
    # aggregation matmul also produces the softmax denominator for free.
    g_r_nat = consts.tile([128, 4 * 129], f16, tag="grnat")
    for c in range(4):
        cs = slice(128 * c, 128 * (c + 1))
        pq = pagg.tile([128, 129], f32, tag="agg")
        nc.tensor.matmul(pq[:, 0:128], s_hT[:, cs], s_WrT[:],
                         start=True, stop=True)
        nc.scalar.copy(g_r_nat[:, 129 * c:129 * c + 128], pq[:, 0:128])
        nc.vector.memset(g_r_nat[:, 129 * c + 128:129 * c + 129], 1.0)

    # ---- main loop: one group of 16 target nodes per iteration ----
    # Stage S(g): relus + score matmuls for group g (emitted in iteration g)
    # Stage E(g): exp + den + dma-transpose     (emitted in iteration g+1)
    # Stage P(g): agg, 1/den, normalize, output (emitted in iteration g+2)
    # visit halves alternately (0,8,1,9,...) so consecutive matmuls hit
    # different PE column strips and weight loads overlap compute
    order = [x for p_ in zip(range(8), range(8, 16)) for x in p_]
    state = {"out_ps": None}

    def escore(g):
        e_ps = ppe.tile([128, N], f32, tag="pe")
        # seed PSUM with lin term + mask boost (one K=24 matmul on this
        # group's block of the preloaded seed tile)
        nc.tensor.matmul(e_ps[:], s_Wlm[:], s_seed[:, N * g:N * (g + 1)],
                         start=True, stop=False, skip_group_check=True)
        for n_, k in enumerate(order):
            i = GSIZE * g + k
            t_t = tpool.tile([H * F, N], f16, tag="t")
            if k in ACT_RELU_KS:
                nc.scalar.activation(t_t[:], g_lT[:], Relu,
                                     bias=g_rT[:, i:i + 1], scale=1.0)
            else:
                # (g_lT + bias) max 0.0, one DVE pass (packed 16-bit mode)
                nc.vector.tensor_scalar(t_t[:], g_lT[:], g_rT[:, i:i + 1],
                                        0.0, add, amax)
            half = 64 * (k // 8)
            w0 = 56 - 8 * (k % 8)
            nc.tensor.matmul(e_ps[half:half + 64, :],
                             s_Aaw[:, w0:w0 + 64], t_t[:],
                             start=False, stop=(n_ >= len(order) - 2),
                             tile_position=(0, half),
                             skip_group_check=True)
        return e_ps

    def expt(e_ps):
        # u = exp(e - 30): unmasked j give exp(e) exactly; masked j
        # underflow fp16 to 0. accum_out yields den = sum_j u, so the
        # reciprocal depends only on ScalarE (never on the PE agg stream).
        u = upool.tile([128, N], f16, tag="u")
        den = denp.tile([128, 1], f32, tag="den")
        nc.scalar.activation(u[:], e_ps[:], Exp, bias=neg30[:],
                             accum_out=den[:])
        rd = denp.tile([128, 1], f32, tag="rden")
        nc.vector.reciprocal(rd[:], den[:])
        # transpose u via the DMA XBAR (one 16x128-tiled instruction):
        # uT[p, c, m] = u[m, 128c+p], so uT[:, c, :] is the aggregation
        # stationary for source-node chunk c.
        uT = utpool.tile([128, 4, 128], f16, tag="ut")
        nc.sync.dma_start_transpose(out=uT[:, :, :], in_=u[:])
        return (uT, rd)

    def post_pe(g, uT):
        # aggT[(il,h), hf] = sum_j u[(il,h), j] * g_r[j, hf]; the ones
        # column of g_r_nat yields den = sum_j u in col 128.
        agg_ps = pagg.tile([128, 129], f32, tag="agg")
        for c in range(4):
            nc.tensor.matmul(agg_ps[:], uT[:, c, :],
                             g_r_nat[:, 129 * c:129 * c + 129],
                             start=(c == 0), stop=(c == 3))
        return agg_ps

    def post_dve(g, agg_ps, rd):
        if g % 8 == 0:
            out_ps_t = pout.tile([128, D], f32, tag="out")
            state["out_ps"] = out_ps_t
        out_ps = state["out_ps"]

        # normalize rows by 1/den, keep only the matching head block
        agg_sb = aggp.tile([128, D], f16, tag="aggsb")
        nc.vector.scalar_tensor_tensor(
            agg_sb[:], agg_ps[:, 0:128], rd[:], s_hm[:], mult, mult)

        # out[16q + il, hf] = sum_h agg_sb[(il,h), hf]; 4 groups
        # accumulate into a 64-row stripe via zero-masked weights.
        q = g % 4
        stripe = 64 * ((g % 8) // 4)
        nc.tensor.matmul(out_ps[stripe:stripe + 64, :],
                         s_R[:, 64 * q:64 * q + 64], agg_sb[:],
                         start=(q == 0), stop=(q == 3))
        if q == 3:
            outb = outp.tile([64, D], f32, tag="outb")
            nc.scalar.copy(outb[:], out_ps[stripe:stripe + 64, :])
            r0 = 64 * (g // 4)
            nc.gpsimd.dma_start(out=d_out[r0:r0 + 64, :], in_=outb[:])

    # S(g) in iteration g, E(g) in g+1, agg(g) in g+4, norm/out(g) in g+5:
    # the exp -> dma-transpose -> agg chain costs ~5us (queue + xbar +
    # ~1.2us DMA semaphore propagation), so agg gets three group-periods
    # of slack, and the reciprocal/normalize always find their agg done
    # (no head-of-line blocking of the next group's relus on VectorE).
    eq = []   # [(g, e_ps)] awaiting E
    tq = []   # [(g, uT)] awaiting agg
    aq = []   # [(g, agg_ps)] awaiting norm/out
    for g in range(NGROUPS):
        e_ps = escore(g)
        if eq:
            ge, pe_ = eq.pop(0)
            tq.append((ge, *expt(pe_)))
        npe = 1 if g < 13 else 2
        ndve = 1 if g < 13 else 2
        while npe > 0 and len(tq) > (3 if g < 13 else 1):
            ga, uT, rd = tq.pop(0)
            aq.append((ga, post_pe(ga, uT), rd))
            npe -= 1
        while ndve > 0 and len(aq) > (2 if g < 13 else 1):
            post_dve(*aq.pop(0))
            ndve -= 1
        eq.append((g, e_ps))

    ge, pe_ = eq.pop(0)
    tq.append((ge, *expt(pe_)))
    while tq or aq:
        if tq:
            ga, uT, rd = tq.pop(0)
            aq.append((ga, post_pe(ga, uT), rd))
        if aq:
            post_dve(*aq.pop(0))


def _host_inputs(h, adj, W_l, W_r, a_w):
    """Build the per-core input maps (pure layout/constant prep)."""
    HF = H * F
    Aaw = np.zeros((HF, H), dtype=np.float32)
    for hh in range(H):
        Aaw[hh * F:(hh + 1) * F, hh] = a_w
    Amask = np.zeros((HF, 120), dtype=np.float32)
    Amask[:, 56:64] = (1.0 - NEG_SLOPE) * Aaw
    Amask = Amask.astype(np.float16)
    Aaw02 = (NEG_SLOPE * Aaw).astype(np.float16)
    Wlm = np.zeros((24, 128), dtype=np.float16)
    for p in range(128):
        Wlm[p % H, p] = 1.0          # lin term: select h
        Wlm[H + p // H, p] = 1.0     # mask boost: select i_local
    Rmask = np.zeros((128, 4 * 64), dtype=np.float16)
    for q in range(4):
        for il in range(GSIZE):
            Rmask[il * H:(il + 1) * H, 64 * q + 16 * q + il] = 1.0
    headmask = np.zeros((128, HF), dtype=np.float32)
    for il in range(GSIZE):
        for hh in range(H):
            headmask[il * H + hh, hh * F:(hh + 1) * F] = 1.0
    WlT = np.ascontiguousarray(W_l.T).astype(np.float16)
    WrT = np.ascontiguousarray(W_r.T).astype(np.float16)

    in_maps = []
    for c in range(NCORES):
        b = c // 2
        i0 = IHALF * (c % 2)
        # Roll the node axis so this core's target nodes sit at positions
        # 0..IHALF-1 (the SPMD program indexes g_rT bias columns by local i).
        # Source-node order is permuted consistently everywhere (softmax and
        # aggregation are permutation-invariant over j).
        in_maps.append({
            "hT": np.ascontiguousarray(np.roll(h[b], -i0, axis=0).T).astype(
                np.float16),
            "WlT": WlT,
            "WrT": WrT,
            "Amask": Amask,
            "Aaw02": Aaw02,
            "Wlm": Wlm,
            "Rmask": Rmask,
            "headmask": headmask,
            "mask30": np.ascontiguousarray(
                (MBIG * np.roll(adj[b, i0:i0 + IHALF, :, 0], -i0, axis=1))
                .reshape(NGROUPS, GSIZE, N).transpose(1, 0, 2)
                .reshape(GSIZE, NGROUPS * N)).astype(np.float16),
        })
    return in_maps


_NC_CACHE = {}
LAST_RESULT = None  # BassKernelResults of the most recent kernel() call


def _get_program():
    if "nc" not in _NC_CACHE:
        _NC_CACHE["nc"] = build_program()
    return _NC_CACHE["nc"]


def kernel(h, adj, W_l, W_r, a_w):
    h = np.asarray(h)
    adj = np.asarray(adj)
    W_l = np.asarray(W_l)
    W_r = np.asarray(W_r)
    a_w = np.asarray(a_w)

    nc = _get_program()
    in_maps = _host_inputs(h, adj, W_l, W_r, a_w)
    res = None
    for attempt in range(3):
        try:
            res = run_bass_kernel_spmd(nc, in_maps, list(range(NCORES)))
            break
        except Exception:
            # the axon-proxied device occasionally reports a transient
            # "unrecoverable" state at process start; it self-heals
            if attempt == 2:
                raise
            import time
            time.sleep(20)
    global LAST_RESULT
    LAST_RESULT = res

    out = np.zeros((B, N, D), dtype=np.float32)
    for c in range(NCORES):
        b = c // 2
        i0 = IHALF * (c % 2)
        out[b, i0:i0 + IHALF, :] = res.results[c]["out"]
    return out
